# revision 8
# baseline (speedup 1.0000x reference)
"""Trainium2 Bass kernel for AffinityMatrixConstructLayer (v3, sharded+AG).

Math (matching the reference's index conventions):
  weight W[b, a] = softplusrelu( sum_d ef2[b,d]*ce[d]*ef1[a,d] )   (b: g2 edge, a: g1 edge)
  M[(i2,i1),(k2,k1)] = sum_{b: h2(b)=i2, t2(b)=k2} sum_{a: h1(a)=i1, t1(a)=k1} W[b, a]
                       + diag(Mp[i2, i1])      with Mp = softplusrelu((x1*cn) @ x2.T)
  cn/ce = tanh(Wn/We @ gw + b)

Sharding / structure:
 - The 1024-dim coeff matvec is sharded 128 rows/core (core c loads only
   Wn/We rows 128c:128c+128 in bf16, lhsT-layout). Each core computes its
   tanh'd coeff slice, reshapes it to [16,16] via one-hot matmuls, and a
   1KB-per-rank AllGather yields the full coeff tile [128,16] whose
   (partition, col) -> feature-index map is absorbed by a host-side
   permutation of the staged x1/x2/ef1/ef2 chunks.
 - Each core only needs the <=48 graph-2 edges whose head lies in its 6
   output block-rows. Host passes one-hot SelT [192,48] (edge compaction)
   and scatter matrices S2c [48, 3*112]; the Kron refactor becomes
     Meb[b,a] -> McT[a,jc] (K=192 GEMM, N=48)
     Pc[jc,(i1,k1)] = McT.T @ B1 (K=192, N=2304)
     out_pair = S2c.T @ Pc      (K=48, N=2304)  x3 pairs
 - All big GEMM operands are bf16 (f32 PSUM accumulation); output is
   written bf16 and upcast on host (tolerance is 2e-2).
 - The 48 i2 block-rows split 6/core; per-i2 column rotation (baked into
   S2c) puts the diagonal at k2rot=0 so the Mp row-add is core-invariant;
   host un-rotates (same scheme as v2).
"""

import sys

for _p in ("/opt/trn_rl_repo", "/root/.axon_site/_ro/trn_rl_repo"):
    if _p not in sys.path:
        sys.path.insert(0, _p)

import numpy as np
import ml_dtypes

import concourse.bass as bass
import concourse.mybir as mybir
from concourse.tile import TileContext
from concourse.bass_utils import run_bass_kernel_spmd

F32 = mybir.dt.float32
BF16 = mybir.dt.bfloat16
AF = mybir.ActivationFunctionType
ALU = mybir.AluOpType

N_CORES = 8
N = 48          # nodes per graph
E = 192         # edges per graph
D = 1024        # feature dim
KC = 8          # contraction chunks of 128
I2P = N // N_CORES          # 6 block-rows per core
JC = 48         # compacted graph-2 edge capacity per core
ROWS = I2P * N              # 288 output rows per core
COLS = N * N                # 2304
NT = [(t * 512, min(COLS, (t + 1) * 512)) for t in range((COLS + 511) // 512)]

_CACHE: dict = {}
LAST_RESULTS = None


def _split_multiwaits(nc):
    """This walrus build encodes at most one sync-wait per instruction.
    Move extra waits onto injected single-wait drains on the same engine
    (engine queues execute in order, so semantics are preserved)."""
    for f in nc.m.functions:
        for blk in f.blocks:
            out = []
            for inst in blk.instructions:
                si = getattr(inst, "sync_info", None)
                if si is not None and si.on_wait and len(si.on_wait) > 1:
                    waits = list(si.on_wait)
                    for w in waits[:-1]:
                        d = mybir.InstDrain(
                            name=nc.get_next_instruction_name(),
                            ins=[], outs=[], bass_is_fusable=False)
                        d.engine = inst.engine
                        d.sync_info = mybir.SyncInfo(on_wait=[w], on_update=[])
                        out.append(d)
                    si.on_wait = waits[-1:]
                out.append(inst)
            try:
                blk.instructions[:] = out
            except TypeError:
                blk.instructions = out


def _build() -> bass.Bass:
    if "nc" in _CACHE:
        return _CACHE["nc"]
    nc = bass.Bass(trn_type="TRN2", num_devices=N_CORES)

    d_wnl = nc.dram_tensor("wnl", [128, D], BF16, kind="ExternalInput")
    d_wel = nc.dram_tensor("wel", [128, D], BF16, kind="ExternalInput")
    d_gwc = nc.dram_tensor("gwc", [128, KC], BF16, kind="ExternalInput")
    d_bnbe = nc.dram_tensor("bnbe", [128, 2], F32, kind="ExternalInput")
    d_ab = nc.dram_tensor("ab", [128, 24], F32, kind="ExternalInput")
    d_x1tp = nc.dram_tensor("x1tp", [128, KC * N], BF16, kind="ExternalInput")
    d_x2tp = nc.dram_tensor("x2tp", [128, KC * N], BF16, kind="ExternalInput")
    d_ef1tp = nc.dram_tensor("ef1tp", [128, KC * E], BF16, kind="ExternalInput")
    d_ef2tp = nc.dram_tensor("ef2tp", [128, KC * E], BF16, kind="ExternalInput")
    d_b1h = nc.dram_tensor("b1h", [E, COLS], BF16, kind="ExternalInput")
    d_selth = nc.dram_tensor("selth", [E, JC], BF16, kind="ExternalInput")
    d_s2ch = nc.dram_tensor("s2ch", [JC, 3 * 112], BF16, kind="ExternalInput")
    d_out = nc.dram_tensor("out", [ROWS, COLS], BF16, kind="ExternalOutput")

    with TileContext(nc) as tc:
        with (
            tc.tile_pool(name="const", bufs=1) as cpool,
            tc.tile_pool(name="scratch", bufs=2) as spool,
            tc.tile_pool(name="orow", bufs=3) as opool,
            tc.tile_pool(name="psmall", bufs=1, space="PSUM") as ps,
            tc.tile_pool(name="ppc", bufs=2, space="PSUM") as ppc,
            tc.tile_pool(name="pfin", bufs=2, space="PSUM") as pfin,
            tc.tile_pool(name="dram", bufs=1, space="DRAM") as dpool,
        ):
            # ---------- critical-path loads (sync queue first) ----------
            wnl = cpool.tile([128, D], BF16, tag="wnl", name="wnl")
            nc.sync.dma_start(out=wnl, in_=d_wnl[:, :])
            wel = cpool.tile([128, D], BF16, tag="wel", name="wel")
            nc.sync.dma_start(out=wel, in_=d_wel[:, :])
            gwc = cpool.tile([128, KC], BF16, tag="gwc", name="gwc")
            nc.sync.dma_start(out=gwc, in_=d_gwc[:, :])
            bnbe = cpool.tile([128, 2], F32, tag="bnbe", name="bnbe")
            nc.sync.dma_start(out=bnbe, in_=d_bnbe[:, :])
            ab = cpool.tile([128, 24], F32, tag="ab", name="ab")
            nc.sync.dma_start(out=ab, in_=d_ab[:, :])

            # big structure loads on the sync queue, after the critical five
            b1h0 = cpool.tile([128, COLS], BF16, tag="b1h0", name="b1h0")
            nc.sync.dma_start(out=b1h0, in_=d_b1h[0:128, :])
            b1h1 = cpool.tile([64, COLS], BF16, tag="b1h1", name="b1h1")
            nc.sync.dma_start(out=b1h1, in_=d_b1h[128:192, :])

            # ---------- sharded coeff matvec on PE ----------
            mv2 = ps.tile([128, 2], F32, tag="pA", name="mv2")
            for k in range(KC):
                ks = slice(k * 128, (k + 1) * 128)
                nc.tensor.matmul(mv2[:, 0:1], wnl[:, ks], gwc[:, k:k + 1],
                                 start=(k == 0), stop=(k == KC - 1))
            for k in range(KC):
                ks = slice(k * 128, (k + 1) * 128)
                nc.tensor.matmul(mv2[:, 1:2], wel[:, ks], gwc[:, k:k + 1],
                                 start=(k == 0), stop=(k == KC - 1))

            # tanh(v) = 1 - 2/(exp(2v + 2b) + 1); bnbe holds 2*b.  All ACT
            # funcs in this kernel stay inside one table set
            # (natural_log_exp_and_others) -- no mid-kernel table loads.
            th = cpool.tile([128, 2], F32, tag="th", name="th")
            et = spool.tile([128, 2], F32, tag="et", name="et")
            for s in range(2):
                nc.scalar.activation(et[:, s:s + 1], mv2[:, s:s + 1],
                                     AF.Exp, scale=2.0, bias=bnbe[:, s:s + 1])
            nc.vector.tensor_scalar_add(et, et, 1.0)
            rt = spool.tile([128, 2], F32, tag="rt", name="rt")
            nc.vector.reciprocal(rt, et)
            nc.vector.tensor_scalar(th, rt, -2.0, 1.0, ALU.mult, ALU.add)
            # reshape [128] -> [16, 8] per coeff via one-hot matmuls
            at = spool.tile([128, 32], F32, tag="at", name="at")
            nc.vector.tensor_scalar_mul(at[:, 0:16], ab[:, 0:16], th[:, 0:1])
            nc.vector.tensor_scalar_mul(at[:, 16:32], ab[:, 0:16], th[:, 1:2])
            rs16 = ps.tile([16, 16], F32, tag="pA", name="rs16")
            nc.tensor.matmul(rs16[:, 0:8], at[:, 0:16], ab[:, 16:24],
                             start=True, stop=True)
            nc.tensor.matmul(rs16[:, 8:16], at[:, 16:32], ab[:, 16:24],
                             start=True, stop=True)
            ag_in_sb = spool.tile([16, 16], F32, tag="agin", name="ag_in_sb")
            nc.scalar.copy(ag_in_sb, rs16)

            # bulk loads on the scalar queue, issued after the AG-critical
            # tanh/copy so they don't delay it (data needed only post-AG)
            ef1tp = cpool.tile([128, KC * E], BF16, tag="ef1tp", name="ef1tp")
            nc.scalar.dma_start(out=ef1tp, in_=d_ef1tp[:, :])
            ef2tp = cpool.tile([128, KC * E], BF16, tag="ef2tp", name="ef2tp")
            nc.scalar.dma_start(out=ef2tp, in_=d_ef2tp[:, :])
            x1tp = cpool.tile([128, KC * N], BF16, tag="x1tp", name="x1tp")
            nc.scalar.dma_start(out=x1tp, in_=d_x1tp[:, :])
            x2tp = cpool.tile([128, KC * N], BF16, tag="x2tp", name="x2tp")
            nc.scalar.dma_start(out=x2tp, in_=d_x2tp[:, :])
            selth0 = cpool.tile([128, JC], BF16, tag="selth0", name="selth0")
            nc.scalar.dma_start(out=selth0, in_=d_selth[0:128, :])
            selth1 = cpool.tile([64, JC], BF16, tag="selth1", name="selth1")
            nc.scalar.dma_start(out=selth1, in_=d_selth[128:192, :])
            s2ch = cpool.tile([JC, 3 * 112], BF16, tag="s2ch", name="s2ch")
            nc.scalar.dma_start(out=s2ch, in_=d_s2ch[:, :])

            # ---------- AllGather coeff (1KB/rank -> 8KB) ----------
            ag_in = dpool.tile([16, 16], F32)
            ag_out = dpool.tile([128, 16], F32)
            nc.gpsimd.dma_start(out=ag_in[:], in_=ag_in_sb)
            nc.gpsimd.collective_compute(
                "AllGather",
                mybir.AluOpType.bypass,
                replica_groups=[list(range(N_CORES))],
                ins=[ag_in.opt()],
                outs=[ag_out.opt()],
            )
            coeff = cpool.tile([128, 16], F32, tag="coeff", name="coeff")
            nc.gpsimd.dma_start(out=coeff, in_=ag_out[:])

            # ---------- edge affinity Meb[b, a] (K = 1024, bf16) ----------
            # W[b, a] = Me_ref[b, a] = sum_d ef1[b,d]*ce[d]*ef2[a,d]
            # (the reference's vec_Me indexing reads Me_ref's rows with the
            #  graph-2 edge index), so the scaled/stationary side is ef1.
            aef1 = cpool.tile([128, KC * E], BF16, tag="aef1", name="aef1")
            meb0 = ps.tile([128, E], F32, tag="pB", name="meb0")
            meb1 = ps.tile([64, E], F32, tag="pC", name="meb1")
            for k in range(KC):
                ke = slice(k * E, (k + 1) * E)
                nc.scalar.activation(aef1[:, ke], ef1tp[:, ke], AF.Copy,
                                     scale=coeff[:, 8 + k:9 + k])
                nc.tensor.matmul(meb0, aef1[:, k * E:k * E + 128],
                                 ef2tp[:, ke],
                                 start=(k == 0), stop=(k == KC - 1))
                nc.tensor.matmul(meb1, aef1[:, k * E + 128:(k + 1) * E],
                                 ef2tp[:, ke],
                                 start=(k == 0), stop=(k == KC - 1))

            # ---------- node affinity (rows rolled so 0:6 = owned i2) ----
            a1 = cpool.tile([128, KC * N], BF16, tag="a1", name="a1")
            an = ps.tile([N, N], F32, tag="pA", name="an")
            for k in range(KC):
                kn = slice(k * N, (k + 1) * N)
                nc.vector.tensor_scalar_mul(a1[:, kn], x1tp[:, kn],
                                            coeff[:, k:k + 1])
                nc.tensor.matmul(an, a1[:, kn], x2tp[:, kn],
                                 start=(k == 0), stop=(k == KC - 1))

            # relu(softplus(x) - 0.5), stable:
            # softplus(x) = relu(x) + ln(1 + exp(-|x|))
            def softplus_relu(src_ap, out_ap):
                p, w = src_ap.shape[0], src_ap.shape[1]
                ab_t = spool.tile([p, w], F32, tag="sp_ab", name="sp_ab")
                nc.scalar.activation(ab_t, src_ap, AF.Abs)
                ex = spool.tile([p, w], F32, tag="sp_ex", name="sp_ex")
                nc.scalar.activation(ex, ab_t, AF.Exp, scale=-1.0)
                ln = spool.tile([p, w], F32, tag="sp_ln", name="sp_ln")
                nc.scalar.activation(ln, ex, AF.Ln, bias=1.0)
                rl = spool.tile([p, w], F32, tag="sp_rl", name="sp_rl")
                nc.scalar.activation(rl, src_ap, AF.Relu)
                pre = spool.tile([p, w], F32, tag="sp_pre", name="sp_pre")
                nc.vector.scalar_tensor_tensor(out=pre, in0=rl, scalar=-0.5,
                                               in1=ln, op0=ALU.add,
                                               op1=ALU.add)
                nc.vector.tensor_scalar_max(out_ap, pre, 0.0)

            mebs0 = cpool.tile([128, E], BF16, tag="mebs0", name="mebs0")
            softplus_relu(meb0, mebs0)
            mebs1 = cpool.tile([64, E], BF16, tag="mebs1", name="mebs1")
            softplus_relu(meb1, mebs1)
            msel = cpool.tile([I2P, N], BF16, tag="msel", name="msel")
            softplus_relu(an[0:I2P, :], msel)
            # move owned Mp rows onto partitions 0 / 64 (SBUF->SBUF DMA)
            mp_rows = []
            for pa in range(I2P // 2):
                mr = cpool.tile([65, N], BF16, tag=f"mr{pa}", name=f"mr{pa}")
                nc.sync.dma_start(out=mr[0:1, :],
                                  in_=msel[2 * pa:2 * pa + 1, :])
                nc.sync.dma_start(out=mr[64:65, :],
                                  in_=msel[2 * pa + 1:2 * pa + 2, :])
                mp_rows.append(mr)

            # ---------- McT[a, jc] = Meb[glob(jc), a] (edge compaction) ---
            mct0 = ps.tile([128, JC], F32, tag="pB", name="mct0")
            nc.tensor.matmul(mct0, mebs0[:, 0:128], selth0,
                             start=True, stop=False)
            nc.tensor.matmul(mct0, mebs1[:, 0:128], selth1,
                             start=False, stop=True)
            mct1 = ps.tile([64, JC], F32, tag="pC", name="mct1")
            nc.tensor.matmul(mct1, mebs0[:, 128:192], selth0,
                             start=True, stop=False)
            nc.tensor.matmul(mct1, mebs1[:, 128:192], selth1,
                             start=False, stop=True)
            mcts0 = cpool.tile([128, JC], BF16, tag="mcts0", name="mcts0")
            nc.scalar.copy(mcts0, mct0)
            mcts1 = cpool.tile([64, JC], BF16, tag="mcts1", name="mcts1")
            nc.vector.tensor_copy(mcts1, mct1)

            # ---------- Pc[jc, (i1,k1)] = McT.T @ B1 ----------
            pc_sb = cpool.tile([JC, COLS], BF16, tag="pc_sb", name="pc_sb")
            for ti, (t0, t1) in enumerate(NT):
                w = t1 - t0
                pp = ppc.tile([JC, 512], F32, tag="pc", name="pp")
                nc.tensor.matmul(pp[:, 0:w], mcts0, b1h0[:, t0:t1],
                                 start=True, stop=False)
                nc.tensor.matmul(pp[:, 0:w], mcts1, b1h1[:, t0:t1],
                                 start=False, stop=True)
                if ti % 2 == 0:
                    nc.vector.tensor_copy(pc_sb[:, t0:t1], pp[:, 0:w])
                else:
                    nc.scalar.copy(pc_sb[:, t0:t1], pp[:, 0:w])

            # ---------- final: out_pair = S2c.T @ Pc, diag add, store -----
            for pa in range(I2P // 2):
                orow = opool.tile([112, COLS], BF16, tag="orow", name="orow")
                for ti, (t0, t1) in enumerate(NT):
                    w = t1 - t0
                    fp = pfin.tile([112, 512], F32, tag="fin", name="fp")
                    nc.tensor.matmul(fp[:, 0:w],
                                     s2ch[:, pa * 112:(pa + 1) * 112],
                                     pc_sb[:, t0:t1], start=True, stop=True)
                    if ti % 2 == 0:
                        nc.vector.tensor_copy(orow[:, t0:t1], fp[:, 0:w])
                    else:
                        nc.scalar.copy(orow[:, t0:t1], fp[:, 0:w])
                for off, i2 in ((0, 2 * pa), (64, 2 * pa + 1)):
                    dg = orow[off:off + 1, 0:COLS:N + 1]
                    nc.vector.tensor_add(dg, dg, mp_rows[pa][off:off + 1, :])
                    nc.sync.dma_start(out=d_out[i2 * N:(i2 + 1) * N, :],
                                      in_=orow[off:off + N, :])

    _split_multiwaits(nc)
    _CACHE["nc"] = nc
    return nc


def _make_in_maps(a):
    bf = ml_dtypes.bfloat16
    q = np.arange(128)
    # feature permutation absorbing the AllGather layout:
    # chunk k, partition q  <->  feature rho = 128*(q//16) + 16*k + (q%16)
    RHO = 128 * (q[:, None] // 16) + 16 * np.arange(KC)[None, :] + \
        (q[:, None] % 16)                                   # [128, KC]

    gw = a["global_weight"].astype(np.float32)
    gwc = np.ascontiguousarray(gw.reshape(KC, 128).T).astype(bf)
    ab = np.zeros((128, 24), np.float32)
    ab[:, 0:16] = (q[:, None] % 16 == np.arange(16)[None, :])
    ab[:, 16:24] = (q[:, None] // 16 == np.arange(8)[None, :])

    def chunked(x):  # [n, 1024] -> [128, KC*n] bf16 with RHO permutation
        t = x.T[RHO]                       # [128, KC, n]
        return np.ascontiguousarray(t.reshape(128, -1)).astype(bf)

    x2tp = chunked(a["x2"])
    ef1tp = chunked(a["ef1"])
    ef2tp = chunked(a["ef2"])

    ei1 = a["edge_index1"].astype(np.int64)
    ei2 = a["edge_index2"].astype(np.int64)
    b1h = np.zeros((E, COLS), np.float32)
    b1h[np.arange(E), ei1[0] * N + ei1[1]] = 1.0
    b1h = b1h.astype(bf)

    def wl(Wfull, c):  # rows 128c:128c+128, lhsT chunk-major layout
        sl = Wfull[128 * c:128 * (c + 1), :].astype(np.float32)
        t = sl.T.reshape(KC, 128, 128).transpose(1, 0, 2)   # [dl, k, rl]
        return np.ascontiguousarray(t.reshape(128, D)).astype(bf)

    in_maps = []
    for c in range(N_CORES):
        edges = np.where(ei2[0] // I2P == c)[0]
        assert len(edges) <= JC, f"core {c}: {len(edges)} edges > JC={JC}"
        selth = np.zeros((E, JC), np.float32)
        selth[edges, np.arange(len(edges))] = 1.0
        s2ch = np.zeros((JC, 3 * 112), np.float32)
        for j, e in enumerate(edges):
            i2g = int(ei2[0, e])
            i2l = i2g - I2P * c
            k2r = (int(ei2[1, e]) - i2g) % N
            s2ch[j, (i2l // 2) * 112 + 64 * (i2l % 2) + k2r] = 1.0
        bnbe = 2.0 * np.stack([a["bn"][128 * c:128 * (c + 1)],
                               a["be"][128 * c:128 * (c + 1)]], axis=1)
        x1roll = np.roll(a["x1"], -I2P * c, axis=0)
        in_maps.append({
            "wnl": wl(a["Wn"], c),
            "wel": wl(a["We"], c),
            "gwc": gwc,
            "bnbe": np.ascontiguousarray(bnbe.astype(np.float32)),
            "ab": ab,
            "x1tp": chunked(x1roll),
            "x2tp": x2tp,
            "ef1tp": ef1tp,
            "ef2tp": ef2tp,
            "b1h": b1h,
            "selth": selth.astype(bf),
            "s2ch": s2ch.astype(bf),
        })
    return in_maps


def kernel(**inputs) -> np.ndarray:
    global LAST_RESULTS
    nc = _build()
    a = {k: np.ascontiguousarray(np.asarray(v)) for k, v in inputs.items()}
    in_maps = _make_in_maps(a)
    res = run_bass_kernel_spmd(nc, in_maps, core_ids=list(range(N_CORES)))
    LAST_RESULTS = res

    parts = []
    for c in range(N_CORES):
        # device rows are [i2l, k2rot, (i1, k1)] with
        # k2g = (k2rot + i2l + 6c) mod 48; want [i2l, i1, (k2g, k1)]
        o = np.asarray(res.results[c]["out"]).astype(np.float32)
        o = o.reshape(I2P, N, N, N).transpose(0, 2, 1, 3)
        o = np.stack([np.roll(o[i], i + I2P * c, axis=1)
                      for i in range(I2P)])
        parts.append(o.reshape(ROWS, COLS))
    return np.concatenate(parts, axis=0).astype(np.float32)


if __name__ == "__main__":
    _build()
    print("build OK")


# revision 15
# speedup vs baseline: 1.6201x; 1.6201x over previous
"""Trainium2 Bass kernel for AffinityMatrixConstructLayer (v4, replicated).

Math (matching the reference's index conventions):
  weight W[b, a] = softplusrelu( sum_d ef1[b,d]*ce[d]*ef2[a,d] )  (b: g2 edge)
  M[(i2,i1),(k2,k1)] = sum_{b: h2(b)=i2, t2(b)=k2} sum_{a: h1(a)=i1, t1(a)=k1}
                       W[b, a]  +  diag(Mp[i2, i1])
  cn/ce = tanh(Wn/We @ gw + bias)

Design:
 - Collectives on this stack cost ~55us end-to-end (measured), so the
   coeff matvec is REPLICATED per core, done on the PE with W as the
   streaming operand: 32 matmuls of N=512 (lhsT = a one-hot-column gw
   chunk, rhs = W^T chunk) accumulate mv as 4 psum rows [4, 512]
   (Wn tile0/1, We tile0/1).  Four PE transposes of [4,128] slices +
   strided DVE copies yield coeff_pre [128, 16] (col j = d-chunk j%8 of
   Wn (j<8) / We (j>=8), partition p = feature 128*(j%8)+p), then the
   exp-form tanh gives coeff [128, 16].  W is staged bf16 (4MB/core,
   the HBM floor) in two halves per W so matmuls chase the DMA.
 - Each core only needs the <=48 graph-2 edges whose head lies in its 6
   output block-rows: host passes one-hot SelT [192,48] and scatter
   S2c [48, 3*112]; the refactor is Meb->McT (N=48) -> Pc = McT.T @ B1
   (K=192, N=2304) -> out_pair = S2c.T @ Pc (K=48, N=2304) x3.
 - B1 [192, 2304] one-hot is built on-device (DVE is idle during the W
   stream); all big GEMM operands bf16 (f32 PSUM), output written bf16
   and upcast on host (tolerance 2e-2).
 - 48 i2 block-rows split 6/core; per-i2 k2 rotation (baked into S2c)
   puts the diagonal at k2rot=0 so the Mp row-add is core-invariant;
   host un-rotates.  ACT funcs stay in one table set (exp/ln/abs/relu).
"""

import sys

for _p in ("/opt/trn_rl_repo", "/root/.axon_site/_ro/trn_rl_repo"):
    if _p not in sys.path:
        sys.path.insert(0, _p)

import numpy as np
import ml_dtypes

import concourse.bass as bass
import concourse.mybir as mybir
from concourse.tile import TileContext
from concourse.masks import make_identity
from concourse.bass_utils import run_bass_kernel_spmd

F32 = mybir.dt.float32
BF16 = mybir.dt.bfloat16
I32 = mybir.dt.int32
AF = mybir.ActivationFunctionType
ALU = mybir.AluOpType

N_CORES = 8
N = 48          # nodes per graph
E = 192         # edges per graph
D = 1024        # feature dim
KC = 8          # contraction chunks of 128
I2P = N // N_CORES          # 6 block-rows per core
JC = 48         # compacted graph-2 edge capacity per core
ROWS = I2P * N              # 288 output rows per core
COLS = N * N                # 2304
NT = [(t * 512, min(COLS, (t + 1) * 512)) for t in range((COLS + 511) // 512)]

_CACHE: dict = {}
LAST_RESULTS = None


def _split_multiwaits(nc):
    """This walrus build encodes at most one sync-wait per instruction.
    Move extra waits onto injected single-wait drains on the same engine
    (engine queues execute in order, so semantics are preserved)."""
    for f in nc.m.functions:
        for blk in f.blocks:
            out = []
            for inst in blk.instructions:
                si = getattr(inst, "sync_info", None)
                if si is not None and si.on_wait and len(si.on_wait) > 1:
                    waits = list(si.on_wait)
                    for w in waits[:-1]:
                        d = mybir.InstDrain(
                            name=nc.get_next_instruction_name(),
                            ins=[], outs=[], bass_is_fusable=False)
                        d.engine = inst.engine
                        d.sync_info = mybir.SyncInfo(on_wait=[w], on_update=[])
                        out.append(d)
                    si.on_wait = waits[-1:]
                out.append(inst)
            try:
                blk.instructions[:] = out
            except TypeError:
                blk.instructions = out


def _build() -> bass.Bass:
    if "nc" in _CACHE:
        return _CACHE["nc"]
    nc = bass.Bass(trn_type="TRN2", num_devices=N_CORES)

    d_wtn = nc.dram_tensor("wtn", [128, KC * D], BF16, kind="ExternalInput")
    d_wte = nc.dram_tensor("wte", [128, KC * D], BF16, kind="ExternalInput")
    d_gwh = nc.dram_tensor("gwh", [128, 128], BF16, kind="ExternalInput")
    d_bnbe = nc.dram_tensor("bnbe", [128, 16], F32, kind="ExternalInput")
    d_ei1 = nc.dram_tensor("ei1", [2, E], I32, kind="ExternalInput")
    d_x1tp = nc.dram_tensor("x1tp", [128, KC * N], BF16, kind="ExternalInput")
    d_x2tp = nc.dram_tensor("x2tp", [128, KC * N], BF16, kind="ExternalInput")
    d_ef1tp = nc.dram_tensor("ef1tp", [128, KC * E], BF16, kind="ExternalInput")
    d_ef2tp = nc.dram_tensor("ef2tp", [128, KC * E], BF16, kind="ExternalInput")
    d_selth = nc.dram_tensor("selth", [E, JC], BF16, kind="ExternalInput")
    d_s2ch = nc.dram_tensor("s2ch", [JC, 3 * 112], BF16, kind="ExternalInput")
    d_out = nc.dram_tensor("out", [ROWS, COLS], BF16, kind="ExternalOutput")

    with TileContext(nc) as tc:
        with (
            tc.tile_pool(name="const", bufs=1) as cpool,
            tc.tile_pool(name="scratch", bufs=2) as spool,
            tc.tile_pool(name="orow", bufs=3) as opool,
            tc.tile_pool(name="pmv", bufs=1, space="PSUM") as pmv,
            tc.tile_pool(name="ptp", bufs=1, space="PSUM") as ptp,
            tc.tile_pool(name="pbig", bufs=1, space="PSUM") as pbig,
            tc.tile_pool(name="ppc", bufs=2, space="PSUM") as ppc,
            tc.tile_pool(name="pfin", bufs=2, space="PSUM") as pfin,
        ):
            # ---------- critical-path loads (sync queue, W last-halves
            # chase first-halves so matvec can start early) ----------
            gwh = cpool.tile([128, 128], BF16, tag="gwh", name="gwh")
            nc.sync.dma_start(out=gwh, in_=d_gwh[:, :])
            bnbe = cpool.tile([128, 16], F32, tag="bnbe", name="bnbe")
            nc.sync.dma_start(out=bnbe, in_=d_bnbe[:, :])
            HW = KC * D // 2
            wtn = cpool.tile([128, KC * D], BF16, tag="wtn", name="wtn")
            wte = cpool.tile([128, KC * D], BF16, tag="wte", name="wte")
            nc.sync.dma_start(out=wtn[:, 0:HW], in_=d_wtn[:, 0:HW])
            nc.sync.dma_start(out=wte[:, 0:HW], in_=d_wte[:, 0:HW])
            nc.sync.dma_start(out=wtn[:, HW:2 * HW], in_=d_wtn[:, HW:2 * HW])
            nc.sync.dma_start(out=wte[:, HW:2 * HW], in_=d_wte[:, HW:2 * HW])

            # ---------- B1 one-hot build on DVE (idle during W stream) ---
            ident = cpool.tile([128, 128], F32, tag="ident", name="ident")
            make_identity(nc, ident)
            iota48 = cpool.tile([128, N], F32, tag="iota48", name="iota48")
            nc.gpsimd.iota(iota48, pattern=[[1, N]], base=0,
                           channel_multiplier=0,
                           allow_small_or_imprecise_dtypes=True)
            ev_tiles = []
            for lo, hi in ((0, 128), (128, 192)):
                t = cpool.tile([hi - lo, 2], F32, tag=f"ev{lo}",
                               name=f"ev{lo}")
                nc.gpsimd.dma_start(
                    out=t, in_=d_ei1[:, lo:hi].rearrange("a b -> b a"))
                ev_tiles.append(t)

            # bulk loads on the idle gpsimd queue (SWDGE); issued early,
            # their ~1MB of HBM traffic barely dents the 4MB W stream
            ef1tp = cpool.tile([128, KC * E], BF16, tag="ef1tp", name="ef1tp")
            nc.gpsimd.dma_start(out=ef1tp, in_=d_ef1tp[:, :])
            ef2tp = cpool.tile([128, KC * E], BF16, tag="ef2tp", name="ef2tp")
            nc.gpsimd.dma_start(out=ef2tp, in_=d_ef2tp[:, :])
            x1tp = cpool.tile([128, KC * N], BF16, tag="x1tp", name="x1tp")
            nc.gpsimd.dma_start(out=x1tp, in_=d_x1tp[:, :])
            x2tp = cpool.tile([128, KC * N], BF16, tag="x2tp", name="x2tp")
            nc.gpsimd.dma_start(out=x2tp, in_=d_x2tp[:, :])
            selth0 = cpool.tile([128, JC], BF16, tag="selth0", name="selth0")
            nc.gpsimd.dma_start(out=selth0, in_=d_selth[0:128, :])
            selth1 = cpool.tile([64, JC], BF16, tag="selth1", name="selth1")
            nc.gpsimd.dma_start(out=selth1, in_=d_selth[128:192, :])
            s2ch = cpool.tile([JC, 3 * 112], BF16, tag="s2ch", name="s2ch")
            nc.gpsimd.dma_start(out=s2ch, in_=d_s2ch[:, :])

            def incid(col, tag):
                tiles = []
                for ci, p in ((0, 128), (1, 64)):
                    ev = ev_tiles[ci][:, col:col + 1]
                    t = cpool.tile([p, N], BF16, tag=f"{tag}{ci}",
                                   name=f"{tag}{ci}")
                    nc.vector.tensor_tensor(t, iota48[0:p, :],
                                            ev.broadcast_to((p, N)),
                                            ALU.is_equal)
                    tiles.append(t)
                return tiles

            G1T = incid(0, "G1T")
            H1T = incid(1, "H1T")
            b1 = []
            for ci, p in ((0, 128), (1, 64)):
                ht = cpool.tile([p, COLS], BF16, tag=f"h1tl{ci}",
                                name=f"h1tl{ci}")
                nc.vector.tensor_copy(
                    ht.rearrange("p (a b) -> p a b", b=N),
                    H1T[ci].unsqueeze(1).broadcast_to((p, N, N)))
                bt = cpool.tile([p, COLS], BF16, tag=f"b1{ci}",
                                name=f"b1{ci}")
                nc.vector.tensor_mul(
                    bt.rearrange("p (a b) -> p a b", b=N),
                    ht.rearrange("p (a b) -> p a b", b=N),
                    G1T[ci].unsqueeze(2).broadcast_to((p, N, N)))
                b1.append(bt)

            # ---------- replicated matvec: mv4 rows = (n0, n1, e0, e1) ---
            mv4 = pmv.tile([4, 512], F32, tag="pA", name="mv4")
            for k in range(KC):
                for r in range(4):
                    w = wtn if r < 2 else wte
                    rhs = w[:, k * D + (r % 2) * 512: k * D + (r % 2) * 512 + 512]
                    lhsT = gwh[:, (k * 4 + r) * 4:(k * 4 + r) * 4 + 4]
                    nc.tensor.matmul(mv4, lhsT, rhs,
                                     start=(k == 0 and r == 0),
                                     stop=(k == KC - 1 and r == 3))
            mv_sb = spool.tile([4, 512], F32, tag="mvsb", name="mv_sb")
            nc.scalar.copy(mv_sb, mv4)

            # transpose mv rows into coeff_pre [128, 16] (col = t + 4r)
            coeff_pre = cpool.tile([128, 16], F32, tag="cpre", name="cpre")
            for t in range(4):
                pt = ptp.tile([128, 4], F32, tag="tp", name=f"pt{t}")
                nc.tensor.transpose(pt, mv_sb[0:4, t * 128:(t + 1) * 128],
                                    ident[0:4, 0:4])
                nc.vector.tensor_copy(coeff_pre[:, t:16:4], pt)

            # tanh(v) = 1 - 2/(exp(2v + 2b) + 1); bnbe holds 2*b
            z2 = spool.tile([128, 16], F32, tag="z2", name="z2")
            nc.vector.scalar_tensor_tensor(out=z2, in0=coeff_pre, scalar=2.0,
                                           in1=bnbe, op0=ALU.mult,
                                           op1=ALU.add)
            et = spool.tile([128, 16], F32, tag="et", name="et")
            nc.scalar.activation(et, z2, AF.Exp)
            nc.vector.tensor_scalar_add(et, et, 1.0)
            rt = spool.tile([128, 16], F32, tag="rt", name="rt")
            nc.vector.reciprocal(rt, et)
            coeff = cpool.tile([128, 16], F32, tag="coeff", name="coeff")
            nc.vector.tensor_scalar(coeff, rt, -2.0, 1.0, ALU.mult, ALU.add)

            # ---------- edge affinity Meb[b, a] (K = 1024, bf16) ----------
            # W[b, a] = sum_d ef1[b,d]*ce[d]*ef2[a,d]: scaled side is ef1.
            # meb_all packs the two b-groups: cols 0:192 rows 0:128, and
            # cols 192:384 rows 0:64 (rows 64:128 there are garbage).
            aef1 = cpool.tile([128, KC * E], BF16, tag="aef1", name="aef1")
            meb0 = pbig.tile([128, E], F32, tag="pB", name="meb0")
            meb1 = pbig.tile([64, E], F32, tag="pC", name="meb1")
            for k in range(KC):
                ke = slice(k * E, (k + 1) * E)
                nc.scalar.activation(aef1[:, ke], ef1tp[:, ke], AF.Copy,
                                     scale=coeff[:, 8 + k:9 + k])
                nc.tensor.matmul(meb0, aef1[:, k * E:k * E + 128],
                                 ef2tp[:, ke],
                                 start=(k == 0), stop=(k == KC - 1))
                nc.tensor.matmul(meb1, aef1[:, k * E + 128:(k + 1) * E],
                                 ef2tp[:, ke],
                                 start=(k == 0), stop=(k == KC - 1))

            # ---------- node affinity (rows rolled so 0:6 = owned i2) ----
            a1 = cpool.tile([128, KC * N], BF16, tag="a1", name="a1")
            an = pmv.tile([N, N], F32, tag="pA", name="an")
            for k in range(KC):
                kn = slice(k * N, (k + 1) * N)
                nc.vector.tensor_scalar_mul(a1[:, kn], x1tp[:, kn],
                                            coeff[:, k:k + 1])
                nc.tensor.matmul(an, a1[:, kn], x2tp[:, kn],
                                 start=(k == 0), stop=(k == KC - 1))

            # relu(softplus(x) - 0.5): softplus = relu(x) + ln(1+exp(-|x|))
            def softplus_relu(src_ap, out_ap):
                p, w = src_ap.shape[0], src_ap.shape[1]
                ab_t = spool.tile([p, w], F32, tag="sp_ab", name="sp_ab")
                nc.scalar.activation(ab_t, src_ap, AF.Abs)
                ex = spool.tile([p, w], F32, tag="sp_ex", name="sp_ex")
                nc.scalar.activation(ex, ab_t, AF.Exp, scale=-1.0)
                ln = spool.tile([p, w], F32, tag="sp_ln", name="sp_ln")
                nc.scalar.activation(ln, ex, AF.Ln, bias=1.0)
                rl = spool.tile([p, w], F32, tag="sp_rl", name="sp_rl")
                nc.scalar.activation(rl, src_ap, AF.Relu)
                pre = spool.tile([p, w], F32, tag="sp_pre", name="sp_pre")
                nc.vector.scalar_tensor_tensor(out=pre, in0=rl, scalar=-0.5,
                                               in1=ln, op0=ALU.add,
                                               op1=ALU.add)
                nc.vector.tensor_scalar_max(out_ap, pre, 0.0)

            mebs0 = cpool.tile([128, E], BF16, tag="mebs0", name="mebs0")
            softplus_relu(meb0, mebs0)
            mebs1 = cpool.tile([64, E], BF16, tag="mebs1", name="mebs1")
            softplus_relu(meb1, mebs1)
            msel = cpool.tile([I2P, N], BF16, tag="msel", name="msel")
            softplus_relu(an[0:I2P, :], msel)
            # move owned Mp rows onto partitions 0 / 64 (SBUF->SBUF DMA)
            mp_rows = []
            for pa in range(I2P // 2):
                mr = cpool.tile([65, N], BF16, tag=f"mr{pa}", name=f"mr{pa}")
                nc.sync.dma_start(out=mr[0:1, :],
                                  in_=msel[2 * pa:2 * pa + 1, :])
                nc.sync.dma_start(out=mr[64:65, :],
                                  in_=msel[2 * pa + 1:2 * pa + 2, :])
                mp_rows.append(mr)

            # ---------- McT[a, jc] = Meb[glob(jc), a] (edge compaction) ---
            mct0 = pbig.tile([128, JC], F32, tag="pB", name="mct0")
            nc.tensor.matmul(mct0, mebs0[:, 0:128], selth0,
                             start=True, stop=False)
            nc.tensor.matmul(mct0, mebs1[:, 0:128], selth1,
                             start=False, stop=True)
            mct1 = pbig.tile([64, JC], F32, tag="pC", name="mct1")
            nc.tensor.matmul(mct1, mebs0[:, 128:192], selth0,
                             start=True, stop=False)
            nc.tensor.matmul(mct1, mebs1[:, 128:192], selth1,
                             start=False, stop=True)
            mcts0 = cpool.tile([128, JC], BF16, tag="mcts0", name="mcts0")
            nc.scalar.copy(mcts0, mct0)
            mcts1 = cpool.tile([64, JC], BF16, tag="mcts1", name="mcts1")
            nc.vector.tensor_copy(mcts1, mct1)

            # ---------- Pc[jc, (i1,k1)] = McT.T @ B1 ----------
            pc_sb = cpool.tile([JC, COLS], BF16, tag="pc_sb", name="pc_sb")
            for ti, (t0, t1) in enumerate(NT):
                w = t1 - t0
                pp = ppc.tile([JC, 512], F32, tag="pc", name="pp")
                nc.tensor.matmul(pp[:, 0:w], mcts0, b1[0][:, t0:t1],
                                 start=True, stop=False)
                nc.tensor.matmul(pp[:, 0:w], mcts1, b1[1][:, t0:t1],
                                 start=False, stop=True)
                if ti % 2 == 0:
                    nc.vector.tensor_copy(pc_sb[:, t0:t1], pp[:, 0:w])
                else:
                    nc.scalar.copy(pc_sb[:, t0:t1], pp[:, 0:w])

            # ---------- final: out_pair = S2c.T @ Pc, diag add, store -----
            for pa in range(I2P // 2):
                orow = opool.tile([112, COLS], BF16, tag="orow", name="orow")
                for ti, (t0, t1) in enumerate(NT):
                    w = t1 - t0
                    fp = pfin.tile([112, 512], F32, tag="fin", name="fp")
                    nc.tensor.matmul(fp[:, 0:w],
                                     s2ch[:, pa * 112:(pa + 1) * 112],
                                     pc_sb[:, t0:t1], start=True, stop=True)
                    if ti % 2 == 0:
                        nc.vector.tensor_copy(orow[:, t0:t1], fp[:, 0:w])
                    else:
                        nc.scalar.copy(orow[:, t0:t1], fp[:, 0:w])
                for off, i2 in ((0, 2 * pa), (64, 2 * pa + 1)):
                    dg = orow[off:off + 1, 0:COLS:N + 1]
                    nc.vector.tensor_add(dg, dg, mp_rows[pa][off:off + 1, :])
                    nc.sync.dma_start(out=d_out[i2 * N:(i2 + 1) * N, :],
                                      in_=orow[off:off + N, :])

    _split_multiwaits(nc)
    _CACHE["nc"] = nc
    return nc


def _make_in_maps(a):
    bf = ml_dtypes.bfloat16
    gw = a["global_weight"].astype(np.float32)
    gwh = np.zeros((128, 128), np.float32)
    for k in range(KC):
        for r in range(4):
            gwh[:, (k * 4 + r) * 4 + r] = gw[128 * k:128 * (k + 1)]
    gwh = gwh.astype(bf)

    def wl(Wfull):  # W^T delta-chunk-major: [128, k*1024 + rho]
        t = Wfull.T.astype(np.float32).reshape(KC, 128, D).transpose(1, 0, 2)
        return np.ascontiguousarray(t.reshape(128, KC * D)).astype(bf)

    wtn = wl(a["Wn"])
    wte = wl(a["We"])

    bnbe = np.zeros((128, 16), np.float32)
    for j in range(16):
        b = a["bn"] if j < 8 else a["be"]
        bnbe[:, j] = 2.0 * b[128 * (j % 8):128 * (j % 8) + 128]

    def chunked(x):  # [n, 1024] -> [128, KC*n] bf16, chunk k = feats 128k+p
        t = x.T.astype(np.float32).reshape(KC, 128, -1).transpose(1, 0, 2)
        return np.ascontiguousarray(t.reshape(128, -1)).astype(bf)

    x2tp = chunked(a["x2"])
    ef1tp = chunked(a["ef1"])
    ef2tp = chunked(a["ef2"])

    ei1 = a["edge_index1"].astype(np.int32)
    ei2 = a["edge_index2"].astype(np.int64)

    in_maps = []
    for c in range(N_CORES):
        edges = np.where(ei2[0] // I2P == c)[0]
        assert len(edges) <= JC, f"core {c}: {len(edges)} edges > JC={JC}"
        selth = np.zeros((E, JC), np.float32)
        selth[edges, np.arange(len(edges))] = 1.0
        s2ch = np.zeros((JC, 3 * 112), np.float32)
        for j, e in enumerate(edges):
            i2g = int(ei2[0, e])
            i2l = i2g - I2P * c
            k2r = (int(ei2[1, e]) - i2g) % N
            s2ch[j, (i2l // 2) * 112 + 64 * (i2l % 2) + k2r] = 1.0
        x1roll = np.roll(a["x1"], -I2P * c, axis=0)
        in_maps.append({
            "wtn": wtn,
            "wte": wte,
            "gwh": gwh,
            "bnbe": bnbe,
            "ei1": ei1,
            "x1tp": chunked(x1roll),
            "x2tp": x2tp,
            "ef1tp": ef1tp,
            "ef2tp": ef2tp,
            "selth": selth.astype(bf),
            "s2ch": s2ch.astype(bf),
        })
    return in_maps


def kernel(**inputs) -> np.ndarray:
    global LAST_RESULTS
    nc = _build()
    a = {k: np.ascontiguousarray(np.asarray(v)) for k, v in inputs.items()}
    in_maps = _make_in_maps(a)
    res = run_bass_kernel_spmd(nc, in_maps, core_ids=list(range(N_CORES)))
    LAST_RESULTS = res

    parts = []
    for c in range(N_CORES):
        # device rows are [i2l, k2rot, (i1, k1)] with
        # k2g = (k2rot + i2l + 6c) mod 48; want [i2l, i1, (k2g, k1)]
        o = np.asarray(res.results[c]["out"]).astype(np.float32)
        o = o.reshape(I2P, N, N, N).transpose(0, 2, 1, 3)
        o = np.stack([np.roll(o[i], i + I2P * c, axis=1)
                      for i in range(I2P)])
        parts.append(o.reshape(ROWS, COLS))
    return np.concatenate(parts, axis=0).astype(np.float32)


if __name__ == "__main__":
    _build()
    print("build OK")
